# revision 26
# baseline (speedup 1.0000x reference)
"""Trainium2 Bass kernel for nn_AttentiveAtlasEncoder (VQ codebook encoder).

Data-parallel over batch across 8 NeuronCores. Weights replicated; the
scalar vq_loss is reduced on host from per-core partial sums.

Pipeline per core (batch shard 4096, chunks of 512, b-blocks of 128):
  MLP chain (PE+ACT, feature-on-partition) -> router softmax (tiny) ->
  VQ distances as one matmul per chart (argmax form: 2<v,c>-|c|^2) ->
  custom DVE scan-argmax (one pass, no onehot) -> codebook row fetch via
  SWDGE dma_gather from a 256B-padded HBM table -> structure filter via
  tile-packed small matmuls in a 4-chart stacked layout -> blends and
  outputs in batch-on-partition layout.
"""

import numpy as np

B = 32768
NCORES = 8
BC = B // NCORES          # 4096 per core
IN_DIM = 64
HID = 256
LAT = 32
NCH = 8
CODES = 256
SCALE = float(np.sqrt(HID))
SF_HID = 16
CHUNK = 512
NCHUNK = BC // CHUNK      # 8
NBB = 4                   # 128-blocks per chunk

_cache = {}

OFF_W1T = 0
OFF_W2T = 256
OFF_WRT = 768
OFF_WVT = 784
OFF_DF = 848
OFF_WS1 = 2896
OFF_WS2 = 2928
OFF_IOTA = 2960
OFF_IOTAR = 3119
OFF_OFFS = 2968
OFF_ID = 2976
OFF_B1 = 3104
OFF_B2 = 3106
OFF_BV = 3108
OFF_BS1 = 3109
OFF_BS2 = 3110
OFF_BR = 3111
CPACK_W = 3127


def _register_argmax():
    from concourse import dve_ops
    from concourse.dve_spec import (
        Spec, Src0, Idx, AluOp, Zero, One, lower, select, scan,
    )
    from concourse.dve_uop import DveOpSpec

    name = "ANT_ARGMAX_SCAN"
    for op in dve_ops.OPS:
        if op.name == name:
            return op

    r = scan(AluOp.MAX, Src0)

    def _ref(in0, in1, c0, c1, c2):
        x = np.asarray(in0, np.float32)
        xf = x.reshape(x.shape[0], -1)
        rm = np.maximum.accumulate(xf, axis=1)
        idx = np.arange(xf.shape[1], dtype=np.float32)[None, :]
        b = np.where(xf >= rm, idx, -1.0).astype(np.float32)
        return b.reshape(x.shape), b.max(axis=1, keepdims=True)

    spec = Spec(
        body=select(Src0 >= r, Idx, Zero - One),
        accum=AluOp.MAX,
        reference=_ref,
    )
    row = max(dve_ops._SUB_OPCODE_FOR_NAME.values()) + 1
    dve_ops._SUB_OPCODE_FOR_NAME[name] = row
    shas = {}
    for ver in ("v3", "v4"):
        shas[ver] = DveOpSpec(
            name=name, opcode=row, uops=lower(spec, ver=ver), rd1_en=False
        ).sha(ver)
    op = dve_ops.DveOp(name, spec, subdim=False, uops_sha=shas)
    dve_ops.OPS.append(op)
    dve_ops.CUSTOM_DVE_SPECS[name] = spec
    return op


def _build():
    import concourse.bass as bass
    import concourse.mybir as mybir
    import concourse.tile as tile
    from concourse import library_config
    from contextlib import ExitStack

    AMAX = _register_argmax()
    f32 = mybir.dt.float32
    i32 = mybir.dt.int32
    i16 = mybir.dt.int16
    AF = mybir.ActivationFunctionType
    ALU = mybir.AluOpType

    nc = bass.Bass()

    # ---- DRAM I/O ----
    d_x = nc.dram_tensor("x_sh", [BC, IN_DIM], f32, kind="ExternalInput")
    d_cpack = nc.dram_tensor("cpack", [128, CPACK_W], f32,
                             kind="ExternalInput")
    d_tbl = nc.dram_tensor("tbl", [NCH * CODES, LAT], f32, kind="ExternalInput")

    o_misc = nc.dram_tensor("o_misc", [BC, 10], i32, kind="ExternalOutput")
    o_zpack = nc.dram_tensor("o_zpack", [BC, 104], f32, kind="ExternalOutput")
    o_znall = nc.dram_tensor("o_znall", [BC, NCH * LAT], f32, kind="ExternalOutput")
    o_vq = nc.dram_tensor("o_vq", [1, 3], f32, kind="ExternalOutput")

    with tile.TileContext(nc) as tc, ExitStack() as ctx:
        cst = ctx.enter_context(tc.tile_pool(name="cst", bufs=1))
        sb = ctx.enter_context(tc.tile_pool(name="sb", bufs=2))
        sb3 = ctx.enter_context(tc.tile_pool(name="sb3", bufs=3))
        sbi = ctx.enter_context(tc.tile_pool(name="sbi", bufs=6))
        psm = ctx.enter_context(tc.tile_pool(name="psm", bufs=2, space="PSUM"))
        psnd = ctx.enter_context(tc.tile_pool(name="psnd", bufs=1, space="PSUM"))
        pstp = ctx.enter_context(tc.tile_pool(name="pstp", bufs=2, space="PSUM"))
        sCst = cst.tile([128, CPACK_W], f32, tag="cpack", name="cpack")
        nc.sync.dma_start(out=sCst[:, :], in_=d_cpack[:, :])
        sW1T = [sCst[0:IN_DIM, OFF_W1T + 128 * m: OFF_W1T + 128 * (m + 1)]
                for m in range(2)]
        sW2T = {(k, m): sCst[:, OFF_W2T + 128 * (2 * k + m):
                             OFF_W2T + 128 * (2 * k + m + 1)]
                for k in range(2) for m in range(2)}
        sWrT = [sCst[:, OFF_WRT + NCH * k: OFF_WRT + NCH * (k + 1)]
                for k in range(2)]
        sWvT = [sCst[:, OFF_WVT + LAT * k: OFF_WVT + LAT * (k + 1)]
                for k in range(2)]
        sDfull = sCst[0:LAT + 1, OFF_DF: OFF_DF + NCH * CODES]
        sWs1s = sCst[:, OFF_WS1: OFF_WS1 + 32]
        sWs2s = sCst[:, OFF_WS2: OFF_WS2 + LAT]
        sIota = sCst[:, OFF_IOTA: OFF_IOTA + NCH]
        sIotaR = sCst[:, OFF_IOTAR: OFF_IOTAR + NCH]
        sOffs = sCst[:, OFF_OFFS: OFF_OFFS + NCH]
        sId = sCst[:, OFF_ID: OFF_ID + 128]
        sb1 = [sCst[:, OFF_B1 + m: OFF_B1 + m + 1] for m in range(2)]
        sb2 = [sCst[:, OFF_B2 + m: OFF_B2 + m + 1] for m in range(2)]
        sBv = sCst[0:LAT, OFF_BV: OFF_BV + 1]
        sBs1s = sCst[:, OFF_BS1: OFF_BS1 + 1]
        sBs2s = sCst[:, OFF_BS2: OFF_BS2 + 1]
        sBr = sCst[0:1, OFF_BR: OFF_BR + NCH]
        sOnes1 = cst.tile([1, 128], f32, tag="ones1", name="ones1")
        nc.vector.memset(sOnes1[:, :], 1.0)
        sOnes128 = cst.tile([128, 1], f32, tag="ones128", name="ones128")
        nc.vector.memset(sOnes128[:, :], 1.0)

        vqA = cst.tile([128, NCHUNK * NBB], f32, tag="vqA", name="vqA")
        vqB = cst.tile([128, NCHUNK * NBB], f32, tag="vqB", name="vqB")
        vqC = cst.tile([128, NCHUNK * NBB], f32, tag="vqC", name="vqC")

        for ci in range(NCHUNK):
            r0 = ci * CHUNK
            # ---------- MLP chain (feature-on-partition, chunk of 512) ----
            xTps = psm.tile([IN_DIM, CHUNK], f32, tag="mlp", name="mlp")
            for bb in range(NBB):
                xb = sb3.tile([128, IN_DIM], f32, tag="xb", name="xb")
                nc.sync.dma_start(
                    out=xb[:, :], in_=d_x[r0 + bb * 128: r0 + (bb + 1) * 128, :])
                nc.tensor.transpose(
                    xTps[:, bb * 128:(bb + 1) * 128], xb[:, :], sId[:, :])
            xTs = sb.tile([IN_DIM, CHUNK], f32, tag="xTs", name="xTs")
            nc.scalar.activation(xTs[:, :], xTps[:, :], AF.Copy)

            h1s = []
            for m in range(2):
                h1ps = psm.tile([128, CHUNK], f32, tag="mlp", name="mlp")
                nc.tensor.matmul(h1ps[:, :], sW1T[m], xTs[:, :],
                                 start=True, stop=True)
                h1t = sb.tile([128, CHUNK], f32, tag=f"h1_{m}", name=f"h1_{m}")
                nc.scalar.activation(h1t[:, :], h1ps[:, :], AF.Gelu,
                                     bias=sb1[m])
                h1s.append(h1t)
            h2s = []
            for m in range(2):
                h2ps = psm.tile([128, CHUNK], f32, tag="mlp", name="mlp")
                nc.tensor.matmul(h2ps[:, :], sW2T[(0, m)], h1s[0][:, :],
                                 start=True, stop=False)
                nc.tensor.matmul(h2ps[:, :], sW2T[(1, m)], h1s[1][:, :],
                                 start=False, stop=True)
                h2t = sb.tile([128, CHUNK], f32, tag=f"h2_{m}", name=f"h2_{m}")
                nc.scalar.activation(h2t[:, :], h2ps[:, :], AF.Gelu,
                                     bias=sb2[m])
                h2s.append(h2t)
            # v (value projection), with ones row appended for the dist matmul
            vps = psm.tile([LAT, CHUNK], f32, tag="mlp", name="mlp")
            nc.tensor.matmul(vps[:, :], sWvT[0], h2s[0][:, :],
                             start=True, stop=False)
            nc.tensor.matmul(vps[:, :], sWvT[1], h2s[1][:, :],
                             start=False, stop=True)
            vtile = sb.tile([LAT + 1, CHUNK], f32, tag="vtile", name="vtile")
            nc.scalar.activation(vtile[0:LAT, :], vps[:, :], AF.Identity,
                                 bias=sBv)
            nc.vector.memset(vtile[LAT:LAT + 1, :], 1.0)

            # ---------- router scores + softmax (no max-sub: |scores|<0.2) --
            scps = psm.tile([128, NBB * NCH], f32, tag="mlp", name="mlp")
            for bb in range(NBB):
                sl = scps[:, bb * NCH:(bb + 1) * NCH]
                nc.tensor.matmul(sl, h2s[0][:, bb * 128:(bb + 1) * 128],
                                 sWrT[0], start=True, stop=False)
                nc.tensor.matmul(sl, h2s[1][:, bb * 128:(bb + 1) * 128],
                                 sWrT[1], start=False, stop=False)
                nc.tensor.matmul(sl, sOnes1[:, :], sBr,
                                 start=False, stop=True)
            Es = sb.tile([128, NBB * NCH], f32, tag="Es", name="Es")
            nc.scalar.activation(Es[:, :], scps[:, :], AF.Exp)
            Ss = sb.tile([128, NBB], f32, tag="Ss", name="Ss")
            nc.vector.tensor_reduce(
                Ss[:, :], Es[:, :].rearrange("p (b n) -> p b n", n=NCH),
                axis=mybir.AxisListType.X, op=ALU.add)
            Rs = sb.tile([128, NBB], f32, tag="Rs", name="Rs")
            nc.vector.reciprocal(Rs[:, :], Ss[:, :])
            Mx = sb.tile([128, NBB], f32, tag="Mx", name="Mx")
            nc.vector.tensor_reduce(
                Mx[:, :], Es[:, :].rearrange("p (b n) -> p b n", n=NCH),
                axis=mybir.AxisListType.X, op=ALU.max)
            rw = sb.tile([128, NBB * NCH], f32, tag="rw", name="rw")
            oh8 = sb.tile([128, NBB * NCH], f32, tag="oh8", name="oh8")
            for bb in range(NBB):
                nc.vector.tensor_scalar(
                    rw[:, bb * NCH:(bb + 1) * NCH],
                    Es[:, bb * NCH:(bb + 1) * NCH],
                    Rs[:, bb:bb + 1], None, op0=ALU.mult)
                nc.vector.tensor_scalar(
                    oh8[:, bb * NCH:(bb + 1) * NCH],
                    Es[:, bb * NCH:(bb + 1) * NCH],
                    Mx[:, bb:bb + 1], None, op0=ALU.is_ge)

            # ---------- VQ per b-block ----------
            zq = sb.tile([128, NBB * NCH, 64], f32, tag="zq", name="zq")
            idxLocs = []
            for bb in range(NBB):
                vaug = vtile[:, bb * 128:(bb + 1) * 128]
                nd = psnd.tile([128, NCH * CODES], f32, tag="nd", name="nd")
                for n in range(NCH):
                    nc.tensor.matmul(
                        nd[:, n * CODES:(n + 1) * CODES], vaug,
                        sDfull[:, n * CODES:(n + 1) * CODES],
                        start=True, stop=True)
                m8 = sb3.tile([128, NCH], f32, tag="m8", name="m8")
                nc.vector.tensor_reduce(
                    m8[:, :],
                    nd[:, :].rearrange("p (n c) -> p n c", c=CODES),
                    axis=mybir.AxisListType.X, op=ALU.max)
                idxU = sb3.tile([128, NCH], mybir.dt.uint32, tag="idxU",
                                name="idxU")
                nc.vector.max_index(idxU[:, :], m8[:, :], nd[:, :])
                idxGF = sb3.tile([128, NCH], f32, tag="idxGF", name="idxGF")
                nc.vector.tensor_copy(idxGF[:, :], idxU[:, :])
                idxLocF = sbi.tile([128, NCH], f32, tag="idxloc",
                                   name="idxloc")
                nc.vector.tensor_tensor(
                    idxLocF[:, :], idxGF[:, :], sOffs, op=ALU.subtract)
                idxLocs.append(idxLocF)
                for n in range(NCH):
                    k = bb * NCH + n
                    nc.gpsimd.indirect_dma_start(
                        out=zq[:, k:k + 1, 0:LAT].rearrange(
                            "p a b -> p (a b)"),
                        out_offset=None,
                        in_=d_tbl[:, :],
                        in_offset=bass.IndirectOffsetOnAxis(
                            ap=idxU[:, n:n + 1], axis=0))

            # ---------- structure filter + blends per b-block ----------
            for bb in range(NBB):
                vtps = pstp.tile([128, LAT], f32, tag="tp", name="tp")
                nc.tensor.transpose(
                    vtps[:, :], vtile[0:LAT, bb * 128:(bb + 1) * 128],
                    sId[0:LAT, 0:LAT])
                vTs = sb.tile([128, LAT], f32, tag="vTs", name="vTs")
                nc.scalar.activation(vTs[:, :], vtps[:, :], AF.Copy)

                # delta = v - z_q in [b, (n,l)] layout, then stack-transpose
                delta = sb.tile([128, NCH * LAT], f32, tag="delta",
                                name="delta")
                for n in range(NCH):
                    k = bb * NCH + n
                    nc.vector.tensor_tensor(
                        delta[:, n * LAT:(n + 1) * LAT], vTs[:, :],
                        zq[:, k:k + 1, 0:LAT].rearrange("p a b -> p (a b)"),
                        op=ALU.subtract)
                dTs = []
                for s in range(2):
                    dTps = pstp.tile([128, 128], f32, tag="tp", name="tp")
                    nc.tensor.transpose(
                        dTps[:, :], delta[:, s * 128:(s + 1) * 128], sId[:, :])
                    dT = sb3.tile([128, 128], f32, tag="dTs", name="dTs")
                    nc.scalar.activation(dT[:, :], dTps[:, :], AF.Copy)
                    dTs.append(dT)

                for s in range(2):
                    g1ps = pstp.tile([128, 128], f32, tag="tp", name="tp")
                    for j in range(4):
                        nc.tensor.matmul(
                            g1ps[j * 32:(j + 1) * 32, :],
                            sWs1s[j * 32:(j + 1) * 32, :],
                            dTs[s][j * 32:(j + 1) * 32, :],
                            start=True, stop=True,
                            tile_position=(j * 32, j * 32))
                    g1s = sb3.tile([128, 128], f32, tag="g1s", name="g1s")
                    nc.scalar.activation(g1s[:, :], g1ps[:, :], AF.Gelu,
                                         bias=sBs1s)
                    znps = pstp.tile([128, 128], f32, tag="tp", name="tp")
                    for j in range(4):
                        nc.tensor.matmul(
                            znps[j * 32:(j + 1) * 32, :],
                            sWs2s[j * 32:j * 32 + SF_HID, :],
                            g1s[j * 32:j * 32 + SF_HID, :],
                            start=True, stop=True,
                            tile_position=(j * 32, j * 32))
                    znTs = sb3.tile([128, 128], f32, tag="znTs", name="znTs")
                    nc.scalar.activation(znTs[:, :], znps[:, :], AF.Identity,
                                         bias=sBs2s)
                    zbps = pstp.tile([128, 128], f32, tag="tp", name="tp")
                    nc.tensor.transpose(zbps[:, :], znTs[:, :], sId[:, :])
                    # write z_n_all into the pad halves of zq rows
                    nc.scalar.activation(
                        zq[:, bb * NCH + s * 4: bb * NCH + (s + 1) * 4,
                           LAT:2 * LAT],
                        zbps[:, :], AF.Copy)

                # z_n_all_charts output
                nc.sync.dma_start(
                    out=o_znall[r0 + bb * 128: r0 + (bb + 1) * 128, :],
                    in_=zq[:, bb * NCH:(bb + 1) * NCH, LAT:2 * LAT])

                # blends: wboth = (zq | z_n_all) * rw_n  -> reduce over n
                wboth = sb.tile([128, NCH, 64], f32, tag="wboth", name="wboth")
                for n in range(NCH):
                    k = bb * NCH + n
                    nc.vector.tensor_scalar(
                        wboth[:, n:n + 1, :], zq[:, k:k + 1, :],
                        rw[:, k:k + 1], None, op0=ALU.mult)
                zpk = sb.tile([128, 136], f32, tag="zpk", name="zpk")
                nc.vector.tensor_reduce(
                    zpk[:, 0:64],
                    wboth[:, :, :].rearrange("p n l -> p l n"),
                    axis=mybir.AxisListType.X, op=ALU.add)
                # zpk: [0:32]=z_q_blended, [32:64]=z_n, [64:96]=z_tex,
                #      [96:128]=z_geo, [128:136]=rw
                nc.vector.tensor_tensor(zpk[:, 96:128], zpk[:, 0:32],
                                        zpk[:, 32:64], op=ALU.add)
                nc.vector.tensor_tensor(zpk[:, 64:96], vTs[:, :],
                                        zpk[:, 96:128], op=ALU.subtract)
                nc.vector.tensor_copy(zpk[:, 128:136],
                                      rw[:, bb * NCH:(bb + 1) * NCH])
                nc.sync.dma_start(
                    out=o_zpack[r0 + bb * 128: r0 + (bb + 1) * 128, :],
                    in_=zpk[:, 32:136])

                # vq loss partials: A=sum v^2, B=sum v*zqb, C=sum_n rw*zq^2
                slot = ci * NBB + bb
                junk32 = sb3.tile([128, LAT], f32, tag="junk32", name="junk32")
                nc.scalar.activation(junk32[:, :], vTs[:, :], AF.Square,
                                     accum_out=vqA[:, slot:slot + 1])
                junkB = sb3.tile([128, LAT], f32, tag="junkB", name="junkB")
                nc.vector.tensor_tensor(junkB[:, :], zpk[:, 0:32], vTs[:, :],
                                        op=ALU.mult)
                nc.vector.tensor_reduce(vqB[:, slot:slot + 1], junkB[:, :],
                                        axis=mybir.AxisListType.X, op=ALU.add)
                junk256 = sb3.tile([128, NCH * LAT], f32, tag="junk256", name="junk256")
                nc.vector.tensor_tensor(
                    junk256[:, :].rearrange("p (n c) -> p n c", c=LAT),
                    wboth[:, :, 0:LAT],
                    zq[:, bb * NCH:(bb + 1) * NCH, 0:LAT], op=ALU.mult)
                nc.vector.tensor_reduce(
                    vqC[:, slot:slot + 1],
                    junk256[:, :].rearrange("p (n c) -> p n c", c=LAT),
                    axis=mybir.AxisListType.XY, op=ALU.add)

                # K_chart / K_code / indices -> packed int32 output
                # first-tie argmax: K = 7 - max(oh8 * (7 - iota))
                KchF = sb.tile([128, 1], f32, tag="kchf", name="kchf")
                KcoF = sb.tile([128, 1], f32, tag="kcof", name="kcof")
                junk8 = sb3.tile([128, NCH], f32, tag="junk8", name="junk8")
                nc.vector.tensor_tensor(
                    junk8[:, :], oh8[:, bb * NCH:(bb + 1) * NCH],
                    sIotaR, op=ALU.mult)
                nc.vector.tensor_reduce(KchF[:, :], junk8[:, :],
                                        axis=mybir.AxisListType.X, op=ALU.max)
                nc.vector.tensor_scalar(KchF[:, :], KchF[:, :], -1.0, 7.0,
                                        op0=ALU.mult, op1=ALU.add)
                oh1 = sb3.tile([128, NCH], f32, tag="oh1", name="oh1")
                nc.vector.tensor_scalar(oh1[:, :], sIota, KchF[:, :],
                                        None, op0=ALU.is_equal)
                junk8b = sb3.tile([128, NCH], f32, tag="junk8b", name="junk8b")
                nc.vector.tensor_tensor(
                    junk8b[:, :], oh1[:, :], idxLocs[bb][:, :], op=ALU.mult)
                nc.vector.tensor_reduce(KcoF[:, :], junk8b[:, :],
                                        axis=mybir.AxisListType.X, op=ALU.add)
                misc = sb.tile([128, 10], i32, tag="misc", name="misc")
                nc.vector.tensor_copy(misc[:, 0:8], idxLocs[bb][:, :])
                nc.vector.tensor_copy(misc[:, 8:9], KchF[:, :])
                nc.vector.tensor_copy(misc[:, 9:10], KcoF[:, :])
                nc.sync.dma_start(
                    out=o_misc[r0 + bb * 128: r0 + (bb + 1) * 128, :],
                    in_=misc[:, :])

        # ---- vq epilogue: per-core partial sums of A, B, C ----
        vqr = cst.tile([128, 3], f32, tag="vqr", name="vqr")
        nc.vector.tensor_reduce(vqr[:, 0:1], vqA[:, :],
                                axis=mybir.AxisListType.X, op=ALU.add)
        nc.vector.tensor_reduce(vqr[:, 1:2], vqB[:, :],
                                axis=mybir.AxisListType.X, op=ALU.add)
        nc.vector.tensor_reduce(vqr[:, 2:3], vqC[:, :],
                                axis=mybir.AxisListType.X, op=ALU.add)
        vqps = pstp.tile([1, 3], f32, tag="tp", name="tp")
        nc.tensor.matmul(vqps[:, :], sOnes128[:, :], vqr[:, :],
                         start=True, stop=True)
        vqout = cst.tile([1, 3], f32, tag="vqout", name="vqout")
        nc.scalar.activation(vqout[:, :], vqps[:, :], AF.Copy)
        nc.sync.dma_start(out=o_vq[:, :], in_=vqout[:, :])

    _split_waits(nc)
    return nc


def _split_waits(nc):
    """This walrus build accepts only one sync-wait per instruction; peel
    extra waits onto preceding same-engine NoOps."""
    import bass_rust
    import concourse.mybir as mybir

    for f in nc.m.functions:
        for b in f.blocks:
            il = b.instructions
            new = []
            for ins in il:
                si = ins.sync_info
                ow = list(si.on_wait) if si is not None and si.on_wait else []
                if len(ow) > 1:
                    for k, w in enumerate(ow[:-1]):
                        new.append(mybir.InstNoOp(
                            name=f"{ins.name}-w{k}", engine=ins.engine,
                            sync_info=bass_rust.SyncInfo(
                                on_wait=[w], on_update=[])))
                    ins.sync_info = bass_rust.SyncInfo(
                        on_wait=[ow[-1]],
                        on_update=list(si.on_update or []))
                new.append(ins)
            il.clear()
            il.extend(new)


def _host_prep(W1, b1, W2, b2, Wk, bk, chart_queries, Wv, bv, codebook,
               Ws1, bs1, Ws2, bs2):
    f = np.float32
    cp = np.zeros((128, CPACK_W), dtype=f)
    cp[0:IN_DIM, OFF_W1T:OFF_W1T + HID] = W1.T
    w2t = W2.T
    for k in range(2):
        for m in range(2):
            cp[:, OFF_W2T + 128 * (2 * k + m): OFF_W2T + 128 * (2 * k + m + 1)] = \
                w2t[k * 128:(k + 1) * 128, m * 128:(m + 1) * 128]
    wrt = (Wk.T @ chart_queries.T) / SCALE
    cp[:, OFF_WRT:OFF_WRT + NCH] = wrt[0:128]
    cp[:, OFF_WRT + NCH:OFF_WRT + 2 * NCH] = wrt[128:256]
    wvt = Wv.T
    cp[:, OFF_WVT:OFF_WVT + LAT] = wvt[0:128]
    cp[:, OFF_WVT + LAT:OFF_WVT + 2 * LAT] = wvt[128:256]
    cb = np.asarray(codebook, dtype=f)
    cp[0:LAT, OFF_DF:OFF_DF + NCH * CODES] = \
        (2.0 * cb).transpose(2, 0, 1).reshape(LAT, NCH * CODES)
    cp[LAT, OFF_DF:OFF_DF + NCH * CODES] = -(cb ** 2).sum(-1).reshape(-1)
    for j in range(4):
        cp[j * 32:(j + 1) * 32, OFF_WS1:OFF_WS1 + SF_HID] = Ws1.T
        cp[j * 32:j * 32 + SF_HID, OFF_WS2:OFF_WS2 + LAT] = Ws2.T
        cp[j * 32:j * 32 + SF_HID, OFF_BS1] = bs1
        cp[j * 32:(j + 1) * 32, OFF_BS2] = bs2
    cp[:, OFF_IOTA:OFF_IOTA + NCH] = np.arange(NCH, dtype=f)[None, :]
    cp[:, OFF_IOTAR:OFF_IOTAR + NCH] = (7.0 - np.arange(NCH, dtype=f))[None, :]
    cp[:, OFF_OFFS:OFF_OFFS + NCH] = (np.arange(NCH, dtype=f) * CODES)[None, :]
    cp[:, OFF_ID:OFF_ID + 128] = np.eye(128, dtype=f)
    cp[:, OFF_B1] = b1[0:128]
    cp[:, OFF_B1 + 1] = b1[128:256]
    cp[:, OFF_B2] = b2[0:128]
    cp[:, OFF_B2 + 1] = b2[128:256]
    cp[0:LAT, OFF_BV] = bv
    cp[0, OFF_BR:OFF_BR + NCH] = (chart_queries @ bk) / SCALE
    tbl = np.ascontiguousarray(cb.reshape(NCH * CODES, LAT))
    return dict(cpack=cp, tbl=tbl)


def kernel(x, W1, b1, W2, b2, Wk, bk, chart_queries, Wv, bv, codebook,
           Ws1, bs1, Ws2, bs2):
    from concourse.bass_utils import run_bass_kernel_spmd

    consts = _host_prep(W1, b1, W2, b2, Wk, bk, chart_queries, Wv, bv,
                        codebook, Ws1, bs1, Ws2, bs2)
    if "nc" not in _cache:
        _cache["nc"] = _build()
    nc = _cache["nc"]

    x = np.ascontiguousarray(np.asarray(x, np.float32))
    in_maps = []
    for c in range(NCORES):
        m = dict(consts)
        m["x_sh"] = np.ascontiguousarray(x[c * BC:(c + 1) * BC, :])
        in_maps.append(m)

    res = run_bass_kernel_spmd(nc, in_maps, core_ids=list(range(NCORES)))
    outs = res.results

    misc = np.concatenate([o["o_misc"] for o in outs], axis=0)
    zpack = np.concatenate([o["o_zpack"] for o in outs], axis=0)
    znall = np.concatenate([o["o_znall"] for o in outs], axis=0)
    vq3 = np.stack([o["o_vq"][0] for o in outs], axis=0)            # [8, 3]

    indices = np.ascontiguousarray(misc[:, 0:8].astype(np.int32))
    K_chart = np.ascontiguousarray(misc[:, 8].astype(np.int32))
    K_code = np.ascontiguousarray(misc[:, 9].astype(np.int32))
    z_n = np.ascontiguousarray(zpack[:, 0:32])
    z_tex = np.ascontiguousarray(zpack[:, 32:64])
    z_geo = np.ascontiguousarray(zpack[:, 64:96])
    router_weights = np.ascontiguousarray(zpack[:, 96:104])
    z_n_all = np.ascontiguousarray(znall.reshape(B, NCH, LAT))
    A, Bt, C = vq3.sum(0)
    commitment = (A - 2.0 * Bt + C) / (B * LAT)
    vq_loss = np.float32(1.25 * commitment)
    return (K_chart, K_code, z_n, z_tex, router_weights, z_geo, vq_loss,
            indices, z_n_all)


# revision 29
# speedup vs baseline: 8157.0564x; 8157.0564x over previous
"""Trainium2 Bass kernel for nn_AttentiveAtlasEncoder (VQ codebook encoder).

Data-parallel over batch across 8 NeuronCores. Weights replicated; the
scalar vq_loss is reduced on host from per-core partial sums.

Pipeline per core (batch shard 4096, chunks of 512, b-blocks of 128):
  MLP chain (PE+ACT, feature-on-partition) -> router softmax (tiny) ->
  VQ distances as one matmul per chart (argmax form: 2<v,c>-|c|^2) ->
  paged reduce_max + max_index (global code ids in two DVE passes) ->
  codebook row fetch via indirect_dma_start row-gather -> structure
  filter via tile_position-packed small matmuls in a 4-chart stacked
  layout -> blends, z_tex/z_geo and vq-loss partials in
  batch-on-partition layout. A post-pass splits multi-wait instructions
  into single-wait NoOp chains for this walrus build.
"""

import numpy as np

B = 32768
NCORES = 8
BC = B // NCORES          # 4096 per core
IN_DIM = 64
HID = 256
LAT = 32
NCH = 8
CODES = 256
SCALE = float(np.sqrt(HID))
SF_HID = 16
CHUNK = 512
NCHUNK = BC // CHUNK      # 8
NBB = 4                   # 128-blocks per chunk

_cache = {}

OFF_W1T = 0
OFF_W2T = 256
OFF_WRT = 768
OFF_WVT = 784
OFF_DF = 848
OFF_WS1 = 2896
OFF_WS2 = 2928
OFF_IOTA = 2960
OFF_IOTAR = 3119
OFF_OFFS = 2968
OFF_ID = 2976
OFF_B1 = 3104
OFF_B2 = 3106
OFF_BV = 3108
OFF_BS1 = 3109
OFF_BS2 = 3110
OFF_BR = 3111
CPACK_W = 3127


def _register_argmax():
    from concourse import dve_ops
    from concourse.dve_spec import (
        Spec, Src0, Idx, AluOp, Zero, One, lower, select, scan,
    )
    from concourse.dve_uop import DveOpSpec

    name = "ANT_ARGMAX_SCAN"
    for op in dve_ops.OPS:
        if op.name == name:
            return op

    r = scan(AluOp.MAX, Src0)

    def _ref(in0, in1, c0, c1, c2):
        x = np.asarray(in0, np.float32)
        xf = x.reshape(x.shape[0], -1)
        rm = np.maximum.accumulate(xf, axis=1)
        idx = np.arange(xf.shape[1], dtype=np.float32)[None, :]
        b = np.where(xf >= rm, idx, -1.0).astype(np.float32)
        return b.reshape(x.shape), b.max(axis=1, keepdims=True)

    spec = Spec(
        body=select(Src0 >= r, Idx, Zero - One),
        accum=AluOp.MAX,
        reference=_ref,
    )
    row = max(dve_ops._SUB_OPCODE_FOR_NAME.values()) + 1
    dve_ops._SUB_OPCODE_FOR_NAME[name] = row
    shas = {}
    for ver in ("v3", "v4"):
        shas[ver] = DveOpSpec(
            name=name, opcode=row, uops=lower(spec, ver=ver), rd1_en=False
        ).sha(ver)
    op = dve_ops.DveOp(name, spec, subdim=False, uops_sha=shas)
    dve_ops.OPS.append(op)
    dve_ops.CUSTOM_DVE_SPECS[name] = spec
    return op


def _build():
    import concourse.bass as bass
    import concourse.mybir as mybir
    import concourse.tile as tile
    from concourse import library_config
    from contextlib import ExitStack

    AMAX = _register_argmax()
    f32 = mybir.dt.float32
    i32 = mybir.dt.int32
    i16 = mybir.dt.int16
    AF = mybir.ActivationFunctionType
    ALU = mybir.AluOpType

    nc = bass.Bass()

    # ---- DRAM I/O ----
    d_x = nc.dram_tensor("x_sh", [BC, IN_DIM], f32, kind="ExternalInput")
    d_cpack = nc.dram_tensor("cpack", [128, CPACK_W], f32,
                             kind="ExternalInput")
    d_tbl = nc.dram_tensor("tbl", [NCH * CODES, LAT], f32, kind="ExternalInput")

    o_misc = nc.dram_tensor("o_misc", [BC, 10], i32, kind="ExternalOutput")
    o_zpack = nc.dram_tensor("o_zpack", [BC, 104], f32, kind="ExternalOutput")
    o_znall = nc.dram_tensor("o_znall", [BC, NCH * LAT], f32, kind="ExternalOutput")
    o_vq = nc.dram_tensor("o_vq", [1, 3], f32, kind="ExternalOutput")

    with tile.TileContext(nc) as tc, ExitStack() as ctx:
        cst = ctx.enter_context(tc.tile_pool(name="cst", bufs=1))
        sb = ctx.enter_context(tc.tile_pool(name="sb", bufs=2))
        sb3 = ctx.enter_context(tc.tile_pool(name="sb3", bufs=3))
        sbi = ctx.enter_context(tc.tile_pool(name="sbi", bufs=6))
        psm = ctx.enter_context(tc.tile_pool(name="psm", bufs=2, space="PSUM"))
        psnd = ctx.enter_context(tc.tile_pool(name="psnd", bufs=1, space="PSUM"))
        pstp = ctx.enter_context(tc.tile_pool(name="pstp", bufs=2, space="PSUM"))
        sCst = cst.tile([128, CPACK_W], f32, tag="cpack", name="cpack")
        nc.sync.dma_start(out=sCst[:, :], in_=d_cpack[:, :])
        sW1T = [sCst[0:IN_DIM, OFF_W1T + 128 * m: OFF_W1T + 128 * (m + 1)]
                for m in range(2)]
        sW2T = {(k, m): sCst[:, OFF_W2T + 128 * (2 * k + m):
                             OFF_W2T + 128 * (2 * k + m + 1)]
                for k in range(2) for m in range(2)}
        sWrT = [sCst[:, OFF_WRT + NCH * k: OFF_WRT + NCH * (k + 1)]
                for k in range(2)]
        sWvT = [sCst[:, OFF_WVT + LAT * k: OFF_WVT + LAT * (k + 1)]
                for k in range(2)]
        sDfull = sCst[0:LAT + 1, OFF_DF: OFF_DF + NCH * CODES]
        sWs1s = sCst[:, OFF_WS1: OFF_WS1 + 32]
        sWs2s = sCst[:, OFF_WS2: OFF_WS2 + LAT]
        sIota = sCst[:, OFF_IOTA: OFF_IOTA + NCH]
        sIotaR = sCst[:, OFF_IOTAR: OFF_IOTAR + NCH]
        sOffs = sCst[:, OFF_OFFS: OFF_OFFS + NCH]
        sId = sCst[:, OFF_ID: OFF_ID + 128]
        sb1 = [sCst[:, OFF_B1 + m: OFF_B1 + m + 1] for m in range(2)]
        sb2 = [sCst[:, OFF_B2 + m: OFF_B2 + m + 1] for m in range(2)]
        sBv = sCst[0:LAT, OFF_BV: OFF_BV + 1]
        sBs1s = sCst[:, OFF_BS1: OFF_BS1 + 1]
        sBs2s = sCst[:, OFF_BS2: OFF_BS2 + 1]
        sBr = sCst[0:1, OFF_BR: OFF_BR + NCH]
        sOnes1 = cst.tile([1, 128], f32, tag="ones1", name="ones1")
        nc.vector.memset(sOnes1[:, :], 1.0)
        sOnes128 = cst.tile([128, 1], f32, tag="ones128", name="ones128")
        nc.vector.memset(sOnes128[:, :], 1.0)

        vqA = cst.tile([128, NCHUNK * NBB], f32, tag="vqA", name="vqA")
        vqB = cst.tile([128, NCHUNK * NBB], f32, tag="vqB", name="vqB")
        vqC = cst.tile([128, NCHUNK * NBB], f32, tag="vqC", name="vqC")

        for ci in range(NCHUNK):
            r0 = ci * CHUNK
            # ---------- MLP chain (feature-on-partition, chunk of 512) ----
            xTps = psm.tile([IN_DIM, CHUNK], f32, tag="mlp", name="mlp")
            for bb in range(NBB):
                xb = sb3.tile([128, IN_DIM], f32, tag="xb", name="xb")
                nc.sync.dma_start(
                    out=xb[:, :], in_=d_x[r0 + bb * 128: r0 + (bb + 1) * 128, :])
                nc.tensor.transpose(
                    xTps[:, bb * 128:(bb + 1) * 128], xb[:, :], sId[:, :])
            xTs = sb.tile([IN_DIM, CHUNK], f32, tag="xTs", name="xTs")
            nc.scalar.activation(xTs[:, :], xTps[:, :], AF.Copy)

            h1s = []
            for m in range(2):
                h1ps = psm.tile([128, CHUNK], f32, tag="mlp", name="mlp")
                nc.tensor.matmul(h1ps[:, :], sW1T[m], xTs[:, :],
                                 start=True, stop=True)
                h1t = sb.tile([128, CHUNK], f32, tag=f"h1_{m}", name=f"h1_{m}")
                nc.scalar.activation(h1t[:, :], h1ps[:, :], AF.Gelu,
                                     bias=sb1[m])
                h1s.append(h1t)
            h2s = []
            for m in range(2):
                h2ps = psm.tile([128, CHUNK], f32, tag="mlp", name="mlp")
                nc.tensor.matmul(h2ps[:, :], sW2T[(0, m)], h1s[0][:, :],
                                 start=True, stop=False)
                nc.tensor.matmul(h2ps[:, :], sW2T[(1, m)], h1s[1][:, :],
                                 start=False, stop=True)
                h2t = sb.tile([128, CHUNK], f32, tag=f"h2_{m}", name=f"h2_{m}")
                nc.scalar.activation(h2t[:, :], h2ps[:, :], AF.Gelu,
                                     bias=sb2[m])
                h2s.append(h2t)
            # v (value projection), with ones row appended for the dist matmul
            vps = psm.tile([LAT, CHUNK], f32, tag="mlp", name="mlp")
            nc.tensor.matmul(vps[:, :], sWvT[0], h2s[0][:, :],
                             start=True, stop=False)
            nc.tensor.matmul(vps[:, :], sWvT[1], h2s[1][:, :],
                             start=False, stop=True)
            vtile = sb.tile([LAT + 1, CHUNK], f32, tag="vtile", name="vtile")
            nc.scalar.activation(vtile[0:LAT, :], vps[:, :], AF.Identity,
                                 bias=sBv)
            nc.vector.memset(vtile[LAT:LAT + 1, :], 1.0)

            # ---------- router scores + softmax (no max-sub: |scores|<0.2) --
            scps = psm.tile([128, NBB * NCH], f32, tag="mlp", name="mlp")
            for bb in range(NBB):
                sl = scps[:, bb * NCH:(bb + 1) * NCH]
                nc.tensor.matmul(sl, h2s[0][:, bb * 128:(bb + 1) * 128],
                                 sWrT[0], start=True, stop=False)
                nc.tensor.matmul(sl, h2s[1][:, bb * 128:(bb + 1) * 128],
                                 sWrT[1], start=False, stop=False)
                nc.tensor.matmul(sl, sOnes1[:, :], sBr,
                                 start=False, stop=True)
            Es = sb.tile([128, NBB * NCH], f32, tag="Es", name="Es")
            nc.scalar.activation(Es[:, :], scps[:, :], AF.Exp)
            Ss = sb.tile([128, NBB], f32, tag="Ss", name="Ss")
            nc.vector.tensor_reduce(
                Ss[:, :], Es[:, :].rearrange("p (b n) -> p b n", n=NCH),
                axis=mybir.AxisListType.X, op=ALU.add)
            Rs = sb.tile([128, NBB], f32, tag="Rs", name="Rs")
            nc.vector.reciprocal(Rs[:, :], Ss[:, :])
            Mx = sb.tile([128, NBB], f32, tag="Mx", name="Mx")
            nc.vector.tensor_reduce(
                Mx[:, :], Es[:, :].rearrange("p (b n) -> p b n", n=NCH),
                axis=mybir.AxisListType.X, op=ALU.max)
            rw = sb.tile([128, NBB * NCH], f32, tag="rw", name="rw")
            oh8 = sb.tile([128, NBB * NCH], f32, tag="oh8", name="oh8")
            for bb in range(NBB):
                nc.vector.tensor_scalar(
                    rw[:, bb * NCH:(bb + 1) * NCH],
                    Es[:, bb * NCH:(bb + 1) * NCH],
                    Rs[:, bb:bb + 1], None, op0=ALU.mult)
                nc.vector.tensor_scalar(
                    oh8[:, bb * NCH:(bb + 1) * NCH],
                    Es[:, bb * NCH:(bb + 1) * NCH],
                    Mx[:, bb:bb + 1], None, op0=ALU.is_ge)

            # ---------- VQ per b-block ----------
            zq = sb.tile([128, NBB * NCH, 64], f32, tag="zq", name="zq")
            idxLocs = []
            for bb in range(NBB):
                vaug = vtile[:, bb * 128:(bb + 1) * 128]
                nd = psnd.tile([128, NCH * CODES], f32, tag="nd", name="nd")
                for n in range(NCH):
                    nc.tensor.matmul(
                        nd[:, n * CODES:(n + 1) * CODES], vaug,
                        sDfull[:, n * CODES:(n + 1) * CODES],
                        start=True, stop=True)
                m8 = sb3.tile([128, NCH], f32, tag="m8", name="m8")
                nc.vector.tensor_reduce(
                    m8[:, :],
                    nd[:, :].rearrange("p (n c) -> p n c", c=CODES),
                    axis=mybir.AxisListType.X, op=ALU.max)
                idxU = sb3.tile([128, NCH], mybir.dt.uint32, tag="idxU",
                                name="idxU")
                nc.vector.max_index(idxU[:, :], m8[:, :], nd[:, :])
                idxGF = sb3.tile([128, NCH], f32, tag="idxGF", name="idxGF")
                nc.vector.tensor_copy(idxGF[:, :], idxU[:, :])
                idxLocF = sbi.tile([128, NCH], f32, tag="idxloc",
                                   name="idxloc")
                nc.vector.tensor_tensor(
                    idxLocF[:, :], idxGF[:, :], sOffs, op=ALU.subtract)
                idxLocs.append(idxLocF)
                for n in range(NCH):
                    k = bb * NCH + n
                    nc.gpsimd.indirect_dma_start(
                        out=zq[:, k:k + 1, 0:LAT].rearrange(
                            "p a b -> p (a b)"),
                        out_offset=None,
                        in_=d_tbl[:, :],
                        in_offset=bass.IndirectOffsetOnAxis(
                            ap=idxU[:, n:n + 1], axis=0))

            # ---------- structure filter + blends per b-block ----------
            for bb in range(NBB):
                vtps = pstp.tile([128, LAT], f32, tag="tp", name="tp")
                nc.tensor.transpose(
                    vtps[:, :], vtile[0:LAT, bb * 128:(bb + 1) * 128],
                    sId[0:LAT, 0:LAT])
                vTs = sb.tile([128, LAT], f32, tag="vTs", name="vTs")
                nc.scalar.activation(vTs[:, :], vtps[:, :], AF.Copy)

                # delta = v - z_q in [b, (n,l)] layout, then stack-transpose
                delta = sb.tile([128, NCH * LAT], f32, tag="delta",
                                name="delta")
                for n in range(NCH):
                    k = bb * NCH + n
                    nc.vector.tensor_tensor(
                        delta[:, n * LAT:(n + 1) * LAT], vTs[:, :],
                        zq[:, k:k + 1, 0:LAT].rearrange("p a b -> p (a b)"),
                        op=ALU.subtract)
                dTs = []
                for s in range(2):
                    dTps = pstp.tile([128, 128], f32, tag="tp", name="tp")
                    nc.tensor.transpose(
                        dTps[:, :], delta[:, s * 128:(s + 1) * 128], sId[:, :])
                    dT = sb3.tile([128, 128], f32, tag="dTs", name="dTs")
                    nc.scalar.activation(dT[:, :], dTps[:, :], AF.Copy)
                    dTs.append(dT)

                for s in range(2):
                    g1ps = pstp.tile([128, 128], f32, tag="tp", name="tp")
                    for j in range(4):
                        nc.tensor.matmul(
                            g1ps[j * 32:(j + 1) * 32, :],
                            sWs1s[j * 32:(j + 1) * 32, :],
                            dTs[s][j * 32:(j + 1) * 32, :],
                            start=True, stop=True,
                            tile_position=(j * 32, j * 32))
                    g1s = sb3.tile([128, 128], f32, tag="g1s", name="g1s")
                    nc.scalar.activation(g1s[:, :], g1ps[:, :], AF.Gelu,
                                         bias=sBs1s)
                    znps = pstp.tile([128, 128], f32, tag="tp", name="tp")
                    for j in range(4):
                        nc.tensor.matmul(
                            znps[j * 32:(j + 1) * 32, :],
                            sWs2s[j * 32:j * 32 + SF_HID, :],
                            g1s[j * 32:j * 32 + SF_HID, :],
                            start=True, stop=True,
                            tile_position=(j * 32, j * 32))
                    znTs = sb3.tile([128, 128], f32, tag="znTs", name="znTs")
                    nc.scalar.activation(znTs[:, :], znps[:, :], AF.Identity,
                                         bias=sBs2s)
                    zbps = pstp.tile([128, 128], f32, tag="tp", name="tp")
                    nc.tensor.transpose(zbps[:, :], znTs[:, :], sId[:, :])
                    # write z_n_all into the pad halves of zq rows
                    nc.scalar.activation(
                        zq[:, bb * NCH + s * 4: bb * NCH + (s + 1) * 4,
                           LAT:2 * LAT],
                        zbps[:, :], AF.Copy)

                # z_n_all_charts output
                nc.sync.dma_start(
                    out=o_znall[r0 + bb * 128: r0 + (bb + 1) * 128, :],
                    in_=zq[:, bb * NCH:(bb + 1) * NCH, LAT:2 * LAT])

                # blends: wboth = (zq | z_n_all) * rw_n  -> reduce over n
                wboth = sb.tile([128, NCH, 64], f32, tag="wboth", name="wboth")
                for n in range(NCH):
                    k = bb * NCH + n
                    nc.vector.tensor_scalar(
                        wboth[:, n:n + 1, :], zq[:, k:k + 1, :],
                        rw[:, k:k + 1], None, op0=ALU.mult)
                zpk = sb.tile([128, 136], f32, tag="zpk", name="zpk")
                nc.vector.tensor_reduce(
                    zpk[:, 0:64],
                    wboth[:, :, :].rearrange("p n l -> p l n"),
                    axis=mybir.AxisListType.X, op=ALU.add)
                # zpk: [0:32]=z_q_blended, [32:64]=z_n, [64:96]=z_tex,
                #      [96:128]=z_geo, [128:136]=rw
                nc.vector.tensor_tensor(zpk[:, 96:128], zpk[:, 0:32],
                                        zpk[:, 32:64], op=ALU.add)
                nc.vector.tensor_tensor(zpk[:, 64:96], vTs[:, :],
                                        zpk[:, 96:128], op=ALU.subtract)
                nc.vector.tensor_copy(zpk[:, 128:136],
                                      rw[:, bb * NCH:(bb + 1) * NCH])
                nc.sync.dma_start(
                    out=o_zpack[r0 + bb * 128: r0 + (bb + 1) * 128, :],
                    in_=zpk[:, 32:136])

                # vq loss partials: A=sum v^2, B=sum v*zqb, C=sum_n rw*zq^2
                slot = ci * NBB + bb
                junk32 = sb3.tile([128, LAT], f32, tag="junk32", name="junk32")
                nc.scalar.activation(junk32[:, :], vTs[:, :], AF.Square,
                                     accum_out=vqA[:, slot:slot + 1])
                junkB = sb3.tile([128, LAT], f32, tag="junkB", name="junkB")
                nc.vector.tensor_tensor(junkB[:, :], zpk[:, 0:32], vTs[:, :],
                                        op=ALU.mult)
                nc.vector.tensor_reduce(vqB[:, slot:slot + 1], junkB[:, :],
                                        axis=mybir.AxisListType.X, op=ALU.add)
                junk256 = sb3.tile([128, NCH * LAT], f32, tag="junk256", name="junk256")
                nc.vector.tensor_tensor(
                    junk256[:, :].rearrange("p (n c) -> p n c", c=LAT),
                    wboth[:, :, 0:LAT],
                    zq[:, bb * NCH:(bb + 1) * NCH, 0:LAT], op=ALU.mult)
                nc.vector.tensor_reduce(
                    vqC[:, slot:slot + 1],
                    junk256[:, :].rearrange("p (n c) -> p n c", c=LAT),
                    axis=mybir.AxisListType.XY, op=ALU.add)

                # K_chart / K_code / indices -> packed int32 output
                # first-tie argmax: K = 7 - max(oh8 * (7 - iota))
                KchF = sb.tile([128, 1], f32, tag="kchf", name="kchf")
                KcoF = sb.tile([128, 1], f32, tag="kcof", name="kcof")
                junk8 = sb3.tile([128, NCH], f32, tag="junk8", name="junk8")
                nc.vector.tensor_tensor(
                    junk8[:, :], oh8[:, bb * NCH:(bb + 1) * NCH],
                    sIotaR, op=ALU.mult)
                nc.vector.tensor_reduce(KchF[:, :], junk8[:, :],
                                        axis=mybir.AxisListType.X, op=ALU.max)
                nc.vector.tensor_scalar(KchF[:, :], KchF[:, :], -1.0, 7.0,
                                        op0=ALU.mult, op1=ALU.add)
                oh1 = sb3.tile([128, NCH], f32, tag="oh1", name="oh1")
                nc.vector.tensor_scalar(oh1[:, :], sIota, KchF[:, :],
                                        None, op0=ALU.is_equal)
                junk8b = sb3.tile([128, NCH], f32, tag="junk8b", name="junk8b")
                nc.vector.tensor_tensor(
                    junk8b[:, :], oh1[:, :], idxLocs[bb][:, :], op=ALU.mult)
                nc.vector.tensor_reduce(KcoF[:, :], junk8b[:, :],
                                        axis=mybir.AxisListType.X, op=ALU.add)
                misc = sb.tile([128, 10], i32, tag="misc", name="misc")
                nc.vector.tensor_copy(misc[:, 0:8], idxLocs[bb][:, :])
                nc.vector.tensor_copy(misc[:, 8:9], KchF[:, :])
                nc.vector.tensor_copy(misc[:, 9:10], KcoF[:, :])
                nc.sync.dma_start(
                    out=o_misc[r0 + bb * 128: r0 + (bb + 1) * 128, :],
                    in_=misc[:, :])

        # ---- vq epilogue: per-core partial sums of A, B, C ----
        vqr = cst.tile([128, 3], f32, tag="vqr", name="vqr")
        nc.vector.tensor_reduce(vqr[:, 0:1], vqA[:, :],
                                axis=mybir.AxisListType.X, op=ALU.add)
        nc.vector.tensor_reduce(vqr[:, 1:2], vqB[:, :],
                                axis=mybir.AxisListType.X, op=ALU.add)
        nc.vector.tensor_reduce(vqr[:, 2:3], vqC[:, :],
                                axis=mybir.AxisListType.X, op=ALU.add)
        vqps = pstp.tile([1, 3], f32, tag="tp", name="tp")
        nc.tensor.matmul(vqps[:, :], sOnes128[:, :], vqr[:, :],
                         start=True, stop=True)
        vqout = cst.tile([1, 3], f32, tag="vqout", name="vqout")
        nc.scalar.activation(vqout[:, :], vqps[:, :], AF.Copy)
        nc.sync.dma_start(out=o_vq[:, :], in_=vqout[:, :])

    _split_waits(nc)
    return nc


def _split_waits(nc):
    """This walrus build accepts only one sync-wait per instruction; peel
    extra waits onto preceding same-engine NoOps."""
    import bass_rust
    import concourse.mybir as mybir

    for f in nc.m.functions:
        for b in f.blocks:
            il = b.instructions
            new = []
            for ins in il:
                si = ins.sync_info
                ow = list(si.on_wait) if si is not None and si.on_wait else []
                if len(ow) > 1:
                    for k, w in enumerate(ow[:-1]):
                        new.append(mybir.InstNoOp(
                            name=f"{ins.name}-w{k}", engine=ins.engine,
                            sync_info=bass_rust.SyncInfo(
                                on_wait=[w], on_update=[])))
                    ins.sync_info = bass_rust.SyncInfo(
                        on_wait=[ow[-1]],
                        on_update=list(si.on_update or []))
                new.append(ins)
            il.clear()
            il.extend(new)


def _host_prep(W1, b1, W2, b2, Wk, bk, chart_queries, Wv, bv, codebook,
               Ws1, bs1, Ws2, bs2):
    f = np.float32
    cp = np.zeros((128, CPACK_W), dtype=f)
    cp[0:IN_DIM, OFF_W1T:OFF_W1T + HID] = W1.T
    w2t = W2.T
    for k in range(2):
        for m in range(2):
            cp[:, OFF_W2T + 128 * (2 * k + m): OFF_W2T + 128 * (2 * k + m + 1)] = \
                w2t[k * 128:(k + 1) * 128, m * 128:(m + 1) * 128]
    wrt = (Wk.T @ chart_queries.T) / SCALE
    cp[:, OFF_WRT:OFF_WRT + NCH] = wrt[0:128]
    cp[:, OFF_WRT + NCH:OFF_WRT + 2 * NCH] = wrt[128:256]
    wvt = Wv.T
    cp[:, OFF_WVT:OFF_WVT + LAT] = wvt[0:128]
    cp[:, OFF_WVT + LAT:OFF_WVT + 2 * LAT] = wvt[128:256]
    cb = np.asarray(codebook, dtype=f)
    cp[0:LAT, OFF_DF:OFF_DF + NCH * CODES] = \
        (2.0 * cb).transpose(2, 0, 1).reshape(LAT, NCH * CODES)
    cp[LAT, OFF_DF:OFF_DF + NCH * CODES] = -(cb ** 2).sum(-1).reshape(-1)
    for j in range(4):
        cp[j * 32:(j + 1) * 32, OFF_WS1:OFF_WS1 + SF_HID] = Ws1.T
        cp[j * 32:j * 32 + SF_HID, OFF_WS2:OFF_WS2 + LAT] = Ws2.T
        cp[j * 32:j * 32 + SF_HID, OFF_BS1] = bs1
        cp[j * 32:(j + 1) * 32, OFF_BS2] = bs2
    cp[:, OFF_IOTA:OFF_IOTA + NCH] = np.arange(NCH, dtype=f)[None, :]
    cp[:, OFF_IOTAR:OFF_IOTAR + NCH] = (7.0 - np.arange(NCH, dtype=f))[None, :]
    cp[:, OFF_OFFS:OFF_OFFS + NCH] = (np.arange(NCH, dtype=f) * CODES)[None, :]
    cp[:, OFF_ID:OFF_ID + 128] = np.eye(128, dtype=f)
    cp[:, OFF_B1] = b1[0:128]
    cp[:, OFF_B1 + 1] = b1[128:256]
    cp[:, OFF_B2] = b2[0:128]
    cp[:, OFF_B2 + 1] = b2[128:256]
    cp[0:LAT, OFF_BV] = bv
    cp[0, OFF_BR:OFF_BR + NCH] = (chart_queries @ bk) / SCALE
    tbl = np.ascontiguousarray(cb.reshape(NCH * CODES, LAT))
    return dict(cpack=cp, tbl=tbl)


def kernel(x, W1, b1, W2, b2, Wk, bk, chart_queries, Wv, bv, codebook,
           Ws1, bs1, Ws2, bs2):
    from concourse.bass_utils import run_bass_kernel_spmd

    consts = _host_prep(W1, b1, W2, b2, Wk, bk, chart_queries, Wv, bv,
                        codebook, Ws1, bs1, Ws2, bs2)
    if "nc" not in _cache:
        _cache["nc"] = _build()
    nc = _cache["nc"]

    x = np.ascontiguousarray(np.asarray(x, np.float32))
    in_maps = []
    for c in range(NCORES):
        m = dict(consts)
        m["x_sh"] = np.ascontiguousarray(x[c * BC:(c + 1) * BC, :])
        in_maps.append(m)

    res = run_bass_kernel_spmd(nc, in_maps, core_ids=list(range(NCORES)))
    outs = res.results

    misc = np.concatenate([o["o_misc"] for o in outs], axis=0)
    zpack = np.concatenate([o["o_zpack"] for o in outs], axis=0)
    znall = np.concatenate([o["o_znall"] for o in outs], axis=0)
    vq3 = np.stack([o["o_vq"][0] for o in outs], axis=0)            # [8, 3]

    indices = np.ascontiguousarray(misc[:, 0:8].astype(np.int32))
    K_chart = np.ascontiguousarray(misc[:, 8].astype(np.int32))
    K_code = np.ascontiguousarray(misc[:, 9].astype(np.int32))
    z_n = np.ascontiguousarray(zpack[:, 0:32])
    z_tex = np.ascontiguousarray(zpack[:, 32:64])
    z_geo = np.ascontiguousarray(zpack[:, 64:96])
    router_weights = np.ascontiguousarray(zpack[:, 96:104])
    z_n_all = np.ascontiguousarray(znall.reshape(B, NCH, LAT))
    A, Bt, C = vq3.sum(0)
    commitment = (A - 2.0 * Bt + C) / (B * LAT)
    vq_loss = np.float32(1.25 * commitment)
    return (K_chart, K_code, z_n, z_tex, router_weights, z_geo, vq_loss,
            indices, z_n_all)
